# revision 64
# baseline (speedup 1.0000x reference)
"""DeepseekV2 MLA decoder-layer attention on 8 Trainium2 NeuronCores.

Distribution (tensor-parallel over heads, per the source hint):
  - A-projection (hidden @ w_qkv_a.T) is sequence-sharded: each core computes
    the fused low-rank latents for its 256-token shard, applies the rmsnorms
    (ln weights folded into the B-projection weights on host) and the k_pe
    RoPE. The kv latents are computed FIRST and AllGathered while the q
    latents are still being computed, so the collectives hide under compute.
  - B-projections, RoPE(q_pe), flash-style causal attention and o_proj are
    head-sharded: core c owns heads {2c, 2c+1}; its o_proj against the matching
    w_o column slice yields a partial [2048, 2048] output.
  - Unshard on host: output = sum of the 8 partials (RowParallel reduction).

Schedule highlights:
  - softmax row-sums accumulate on the Vector engine (exsum), leaving the PE
    one cheap [1,512] matmul per (block, head) instead of one per key tile;
  - rmsnorm scales use a single fused Rsqrt activation (table prewarmed at
    t=0) and the ss row-sum matmuls are emitted one chunk late, so the PE
    never waits on the Vector/Scalar chain mid-phase-1;
  - AllGather payloads stream out per chunk; gathered latents stream back as
    one contiguous DMA per peer core (no strided descriptor storms);
  - both heads' rope columns are packed into one 128-row q_pe projection;
  - diagonal score tiles only compute the causally-valid query columns;
  - softmax normalization and o_proj are emitted one flash iteration behind,
    so their PE instructions never wait on the Vector reciprocal chain.

Layout conventions on device (partition dim first):
  activations feature-major [d, s] so matmuls contract on partitions;
  v is token-major [t, (h, vdim)] so PV contracts over keys;
  scores are computed transposed [t_chunk, s_block].
"""
import numpy as np

import concourse.bass as bass
import concourse.mybir as mybir
import concourse.tile as tile
from concourse import bacc
from concourse.bass_utils import run_bass_kernel_spmd

HIDDEN = 2048
H = 16
NOPE = 128
ROPE = 64
VDIM = 128
QLR = 1536
KVLR = 512
QK = NOPE + ROPE            # 192
THETA = 10000.0
EPS = 1e-6
SEQ = 2048

N_CORES = 8
HPC = H // N_CORES          # 2 heads per core
SSH = SEQ // N_CORES        # 256-token shard
LAT_C = 17                  # latent chunks: 12 q_a + 4 kv_a + 1 (kpe, 64 rows)
P = 128

F32 = mybir.dt.float32
F32R = mybir.dt.float32r
BF16 = mybir.dt.bfloat16
F16 = mybir.dt.float16
FLASH_DT = F16              # dtype of q/k/v/exp inside flash attention
LAT_DT = F16                # dtype of the AllGather payload + B-proj operands
EXP_BIAS = -4.0             # exp(x*scale + EXP_BIAS): cancels in softmax ratio,
                            # keeps fp16 exp values in range

SCALE = float(QK) ** -0.5
NEG = -60000.0              # "-inf" that still fits in fp16

N_KC = HIDDEN // P          # 16
N_QAC = QLR // P            # 12
N_KVC = KVLR // P           # 4
N_KVG = LAT_C - N_QAC       # 5 gathered kv chunks (4 kv_a + kpe)
N_SB = SEQ // 512           # 4 query blocks
N_SC = SEQ // P             # 16


def build_program():
    nc = bacc.Bacc("TRN2", target_bir_lowering=False, debug=False,
                   num_devices=N_CORES)

    h1 = nc.dram_tensor("h1", [P, N_KC, SSH], F16, kind="ExternalInput")
    w1 = nc.dram_tensor("w1", [LAT_C, P, HIDDEN], F16, kind="ExternalInput")
    wq = nc.dram_tensor("wq", [P, N_QAC, HPC * QK], LAT_DT, kind="ExternalInput")
    wkv = nc.dram_tensor("wkv", [P, N_KVC, HPC * (NOPE + VDIM)], LAT_DT, kind="ExternalInput")
    wo = nc.dram_tensor("wo", [P, HPC, HIDDEN], F16, kind="ExternalInput")
    cosq = nc.dram_tensor("cosq", [P, SEQ], F16, kind="ExternalInput")
    ssinq = nc.dram_tensor("ssinq", [P, SEQ], F16, kind="ExternalInput")
    cosl = nc.dram_tensor("cosl", [ROPE, SSH], F16, kind="ExternalInput")
    ssinl = nc.dram_tensor("ssinl", [ROPE, SSH], F16, kind="ExternalInput")
    pswap = nc.dram_tensor("pswap", [P, P], F16, kind="ExternalInput")
    onesc_d = nc.dram_tensor("onesc", [P, 1], F32R, kind="ExternalInput")
    onesr_d = nc.dram_tensor("onesr", [1, P], F32R, kind="ExternalInput")
    yout = nc.dram_tensor("y", [SEQ, HIDDEN], F16, kind="ExternalOutput")

    with tile.TileContext(nc) as tc:
        _emit(nc, tc, h1, w1, wq, wkv, wo, cosq, ssinq, cosl, ssinl, pswap,
              onesc_d, onesr_d, yout)
    nc.compile()
    return nc


def _emit(nc, tc, h1, w1, wq, wkv, wo, cosq, ssinq, cosl, ssinl, pswap,
          onesc_d, onesr_d, yout):
    Exp = mybir.ActivationFunctionType.Exp
    Sqrt = mybir.ActivationFunctionType.Sqrt
    rg = [list(range(N_CORES))]

    with tc.tile_pool(name="const", bufs=1) as const, \
         tc.tile_pool(name="work", bufs=2) as work, \
         tc.tile_pool(name="lstr", bufs=2) as lstr, \
         tc.tile_pool(name="epool", bufs=6) as epool, \
         tc.tile_pool(name="psum", bufs=1, space="PSUM") as psum, \
         tc.tile_pool(name="dram", bufs=1, space="DRAM") as dram:

        # PSUM tags (8 banks): qacc x4, oacc, oacc2, bcast, zacc
        def ps(shape, tag, name, bufs=None):
            return psum.tile(shape, F32, tag=tag, name=name, bufs=bufs)

        # ---- constants ----
        ones_col_t = const.tile([P, 1], F32R)
        nc.sync.dma_start(ones_col_t[:], onesc_d[:])
        ones_col = ones_col_t[:]
        ones_row_t = const.tile([1, P], F32R)
        nc.sync.dma_start(ones_row_t[:], onesr_d[:])
        ones_row = ones_row_t[:]
        psw = const.tile([P, P], F16)
        nc.sync.dma_start(psw[:], pswap[:])
        eps1 = const.tile([1, 1], F32)
        nc.vector.memset(eps1[:], EPS)
        negc = const.tile([P, 1], F32)
        nc.vector.memset(negc[:], EXP_BIAS)
        warm = work.tile([1, 1], F32, name="warm", bufs=1)
        nc.scalar.activation(warm[:], eps1[:], Sqrt, bias=eps1[:], scale=1.0)


        with tc.tile_pool(name="att", bufs=1) as att:
            # persistent attention-phase tiles (DMAs issued mid-phase-1 so the
            # weights land before their consumers without starving w1/h1)
            wqs = att.tile([P, N_QAC, HPC * QK], LAT_DT)
            wkvs = att.tile([P, N_KVC, HPC * (NOPE + VDIM)], LAT_DT)
            wos = att.tile([P, HPC, HIDDEN], F16)
            qn = [att.tile([P, SEQ], FLASH_DT, name=f"qn{h}") for h in range(HPC)]
            qpk = att.tile([P, SEQ], F16)       # packed q_pe (h0 rows 0:64, h1 64:128)
            qpb = [att.tile([ROPE, SEQ], FLASH_DT, name=f"qpb{h}") for h in range(HPC)]
            kn = [att.tile([P, SEQ], FLASH_DT, name=f"kn{h}") for h in range(HPC)]
            kpe_sb = att.tile([ROPE, SEQ], FLASH_DT)
            vv = att.tile([P, N_SC, HPC * VDIM], FLASH_DT)
            ao = [att.tile([P, SEQ], FLASH_DT, name=f"ao{h}") for h in range(HPC)]

            ag_in1 = dram.tile([P, N_KVG * SSH], LAT_DT)
            ag_out1 = dram.tile([N_CORES, P, N_KVG * SSH], LAT_DT,
                                addr_space="Shared")
            ag_in2 = dram.tile([P, N_QAC * SSH], LAT_DT)
            ag_out2 = dram.tile([N_CORES, P, N_QAC * SSH], LAT_DT,
                                addr_space="Shared")

            # ============ phase 1: local A-proj + rmsnorm + kpe rope ========
            with tc.tile_pool(name="ph1", bufs=1) as ph1:
                hloc = ph1.tile([P, N_KC, SSH], F16)
                lat = ph1.tile([P, LAT_C, SSH], F16)
                latf = ph1.tile([P, LAT_C, SSH], LAT_DT)

                # first matmul's operands lead the DMA queues (wt0 split so it
                # spreads over 4 queues and lands fast; h1 grouped 4 chunks
                # per DMA for big contiguous descriptors)
                nc.sync.dma_start(hloc[:, 0:2, :], h1[:, 0:2, :])
                # first chunk's weights as 4 independent quarter tiles so the
                # very first matmuls start as soon as the first quarter lands
                wt0q = [ph1.tile([P, 512], F16, name="w1q", bufs=4)
                        for _ in range(4)]
                for j in range(4):
                    nc.sync.dma_start(wt0q[j][:], w1[16][:, j * 512:(j + 1) * 512])
                nc.sync.dma_start(hloc[:, 2:4, :], h1[:, 2:4, :])
                for g in range(2, 8):
                    nc.sync.dma_start(hloc[:, 2 * g:2 * (g + 1), :],
                                      h1[:, 2 * g:2 * (g + 1), :])

                pend_pe = []

                def flush_pe():
                    while pend_pe:
                        pend_pe.pop(0)()

                def a_chunk(m, wt, ss=None, ss_start=False, ss_stop=False):
                    acc = ps([P, SSH], "qacc", "a_acc", bufs=4)
                    for k in range(N_KC):
                        if isinstance(wt, list):
                            wsl = wt[k // 4][:, (k % 4) * P:(k % 4 + 1) * P]
                        else:
                            wsl = wt[:, k * P:(k + 1) * P]
                        nc.tensor.matmul(acc[:], wsl, hloc[:, k, :],
                                         start=(k == 0), stop=(k == N_KC - 1))
                    # deferred PE work whose vector inputs are ready by now
                    flush_pe()
                    with nc.allow_low_precision(reason="fp16 latent staging"):
                        nc.vector.tensor_copy(lat[:, m, :], acc[:])
                    if ss is not None:
                        sq = work.tile([P, SSH], F32R, name="sq")
                        nc.vector.tensor_mul(sq[:], lat[:, m, :], lat[:, m, :])

                        def ssmm(ss=ss, sq=sq, st=ss_start, sp=ss_stop):
                            nc.tensor.matmul(ss[:], ones_col[:], sq[:],
                                             start=st, stop=sp)
                        pend_pe.append(ssmm)

                def compute_scale(ss, denom, tag):
                    rt = work.tile([1, SSH], F32, name=f"rt{tag}", bufs=1)
                    nc.scalar.activation(rt[:], ss[:], Sqrt, bias=eps1[:],
                                         scale=1.0 / denom)
                    ri = work.tile([1, SSH], F32R, name=f"ri{tag}", bufs=1)
                    with nc.allow_low_precision(reason="float32r is bitwise float32"):
                        nc.vector.reciprocal(ri[:], rt[:])
                    return ri

                def apply_scale(ri, mbase, nchunk, tag, agt):
                    bc = ps([P, SSH], "bcast", f"bc{tag}")
                    nc.tensor.matmul(bc[:], ones_row[:], ri[:], start=True, stop=True)
                    bcs = work.tile([P, SSH], F32, name=f"bcs{tag}", bufs=1)
                    nc.vector.tensor_copy(bcs[:], bc[:])
                    for m in range(mbase, mbase + nchunk):
                        with nc.allow_low_precision(reason="fp16 AllGather payload"):
                            nc.vector.tensor_mul(latf[:, m, :],
                                                 lat[:, m, :], bcs[:])
                        o = m - mbase
                        if agt is ag_in1:
                            # latency-critical: spray across two queues
                            hs = SSH // 2
                            for u in range(2):
                                nc.sync.dma_start(
                                    agt[:, o * SSH + u * hs:o * SSH + (u + 1) * hs],
                                    latf[:, m, u * hs:(u + 1) * hs])
                        elif m % 2 == 1:
                            # AG2 is CC-gated anyway: pair chunks, fewer issues
                            nc.sync.dma_start(
                                agt[:, (o - 1) * SSH:(o + 1) * SSH],
                                latf[:, m - 1:m + 1, :])

                # --- 1a: kpe + kv chunks, then AllGather them ---
                ss_kv = ps([1, SSH], "zacc", "ss_kv")
                order1a = [16, 12, 13, 14, 15]
                for i, m in enumerate(order1a):
                    if i == 0:
                        wt = wt0q
                    else:
                        wt = ph1.tile([P, HIDDEN], F16, name="w1t", bufs=4)
                        eng = nc.scalar if i % 2 else nc.sync
                        eng.dma_start(wt[:], w1[m])
                    a_chunk(m, wt, ss=None if m == 16 else ss_kv,
                            ss_start=(m == 12), ss_stop=(m == 15))

                # rope on local k_pe (chunk 16, rows 0:64)
                cl = work.tile([ROPE, 2, SSH], F16, name="cl", bufs=1)
                nc.sync.dma_start(cl[:, 0, :], cosl[:])
                nc.sync.dma_start(cl[:, 1, :], ssinl[:])
                swp = ps([ROPE, SSH], "qacc", "swp", bufs=4)
                nc.tensor.matmul(swp[:], psw[:ROPE, :ROPE], lat[:ROPE, 16, :],
                                 start=True, stop=True)
                sws = work.tile([ROPE, SSH], F32R, name="sws", bufs=1)
                nc.vector.tensor_mul(sws[:], swp[:], cl[:, 1, :])
                t1 = work.tile([ROPE, SSH], F32R, name="t1", bufs=1)
                nc.vector.tensor_mul(t1[:], lat[:ROPE, 16, :], cl[:, 0, :])
                with nc.allow_low_precision(reason="fp16 AllGather payload"):
                    nc.vector.tensor_add(latf[:ROPE, 16, :], t1[:], sws[:])
                    nc.vector.memset(latf[ROPE:, 16, :], 0.0)
                for u in range(2):
                    hs = SSH // 2
                    nc.sync.dma_start(
                        ag_in1[:, 4 * SSH + u * hs:4 * SSH + (u + 1) * hs],
                        latf[:, 16, u * hs:(u + 1) * hs])

                flush_pe()
                ri_kv = compute_scale(ss_kv, KVLR, "kv")
                pend_pe.append(
                    lambda: apply_scale(ri_kv, N_QAC, N_KVC, "kv", ag_in1))

                # --- 1b: q chunks, then AllGather them ---
                ss_q = ps([1, SSH], "zacc", "ss_q")
                for m in range(N_QAC):
                    if m == 0:
                        # kv scale + AllGather-input DMAs go ahead of 1b's
                        # w1 bytes in the queues
                        flush_pe()
                    wt = ph1.tile([P, HIDDEN], F16, name="w1t", bufs=4)
                    eng = nc.scalar if m % 2 else nc.sync
                    eng.dma_start(wt[:], w1[m])
                    a_chunk(m, wt, ss=ss_q,
                            ss_start=(m == 0), ss_stop=(m == N_QAC - 1))
                    if m == 0:
                        # kv payload is on the wire: fire its AllGather
                        nc.gpsimd.collective_compute(
                            "AllGather", mybir.AluOpType.bypass,
                            replica_groups=rg,
                            ins=[ag_in1.opt()], outs=[ag_out1.opt()],
                        )
                    elif m == 1:
                        # B-projection weights ride on the Act DGE
                        nc.scalar.dma_start(wkvs[:], wkv[:])
                        nc.scalar.dma_start(wqs[:], wq[:])
                flush_pe()
                ri_q = compute_scale(ss_q, QLR, "q")
                apply_scale(ri_q, 0, N_QAC, "q", ag_in2)
                nc.gpsimd.collective_compute(
                    "AllGather", mybir.AluOpType.bypass, replica_groups=rg,
                    ins=[ag_in2.opt()], outs=[ag_out2.opt()],
                )
                # prewarm the Exp table while the scalar engine is idle
                nc.scalar.activation(warm[:], eps1[:], Exp, bias=eps1[:],
                                     scale=1.0)

            # ================= phase 3: B-projections =================
            assert FLASH_DT is LAT_DT

            # kv-projection: latents stream back as one contiguous DMA per
            # peer core; matmuls take a strided 2-free-dim rhs view so they
            # stay 512-wide
            for nb in range(N_SB):
                c0 = nb * 2
                sblk = slice(nb * 512, (nb + 1) * 512)
                lkv = lstr.tile([P, 2, N_KVG * SSH], LAT_DT, name="lkv", bufs=2)
                for i in range(2):
                    w = N_KVG * SSH // 4
                    for u in range(4):
                        nc.sync.dma_start(lkv[:, i, u * w:(u + 1) * w],
                                          ag_out1[c0 + i][:, u * w:(u + 1) * w])
                lkvv = lkv[:].rearrange("p c (m s) -> p c m s", m=N_KVG)
                for i in range(2):
                    col = nb * 512 + 256 * i
                    with nc.allow_low_precision(reason="flash operands are fp16"):
                        nc.vector.tensor_copy(kpe_sb[:, col:col + SSH],
                                              lkvv[:ROPE, i, 4, :])
                for h in range(HPC):
                    acc = ps([P, 512], "qacc", "kn_acc", bufs=4)
                    for k in range(N_KVC):
                        nc.tensor.matmul(acc[:], wkvs[:, k, h * NOPE:(h + 1) * NOPE],
                                         lkvv[:, :, k, :],
                                         start=(k == 0), stop=(k == N_KVC - 1))
                    with nc.allow_low_precision(reason="flash operands are fp16"):
                        nc.vector.tensor_copy(kn[h][:, sblk], acc[:])
                for i in range(2):
                    for tsub in range(2):
                        t_idx = nb * 4 + i * 2 + tsub
                        acc = ps([P, HPC * VDIM], "qacc", "v_acc", bufs=4)
                        for k in range(N_KVC):
                            nc.tensor.matmul(
                                acc[:], lkvv[:, i, k, tsub * P:(tsub + 1) * P],
                                wkvs[:, k, HPC * NOPE:],
                                start=(k == 0), stop=(k == N_KVC - 1))
                        with nc.allow_low_precision(reason="flash operands are fp16"):
                            nc.vector.tensor_copy(vv[:, t_idx, :], acc[:])

            # o_proj weights + rope tables load in the dead-queue window while
            # the second AllGather is on the wire
            for u in range(4):
                nc.sync.dma_start(wos[:, :, u * 512:(u + 1) * 512],
                                  wo[:, :, u * 512:(u + 1) * 512])
            cqts = []
            for nb in range(N_SB):
                cqt = work.tile([P, 2, 512], F16, name="cqt", bufs=4)
                nc.sync.dma_start(cqt[:, 0, :], cosq[:, nb * 512:(nb + 1) * 512])
                nc.sync.dma_start(cqt[:, 1, :], ssinq[:, nb * 512:(nb + 1) * 512])
                cqts.append(cqt)

            # q-projection: k-outer; both heads' rope columns packed into one
            # 128-row accumulator (wq host layout: [n0|n1|rope0:rope1])
            for nb in range(N_SB):
                c0 = nb * 2
                sblk = slice(nb * 512, (nb + 1) * 512)
                lq = lstr.tile([P, 2, N_QAC * SSH], LAT_DT, name="lq", bufs=2)
                for i in range(2):
                    w = N_QAC * SSH // 3
                    for u in range(3):
                        nc.sync.dma_start(lq[:, i, u * w:(u + 1) * w],
                                          ag_out2[c0 + i][:, u * w:(u + 1) * w])
                lqv = lq[:].rearrange("p c (m s) -> p c m s", m=N_QAC)
                an0 = ps([P, 512], "oacc", "qn0_acc")
                an1 = ps([P, 512], "oacc2", "qn1_acc")
                arp = ps([P, 512], "bcast", "qrp_acc")
                for k in range(N_QAC):
                    st, sp = (k == 0), (k == N_QAC - 1)
                    qa = lqv[:, :, k, :]
                    nc.tensor.matmul(an0[:], wqs[:, k, 0:NOPE], qa,
                                     start=st, stop=sp)
                    nc.tensor.matmul(an1[:], wqs[:, k, NOPE:2 * NOPE], qa,
                                     start=st, stop=sp)
                    nc.tensor.matmul(arp[:], wqs[:, k, 2 * NOPE:], qa,
                                     start=st, stop=sp)
                with nc.allow_low_precision(reason="flash operands are fp16"):
                    nc.vector.tensor_copy(qn[0][:, sblk], an0[:])
                    nc.vector.tensor_copy(qn[1][:, sblk], an1[:])
                    nc.vector.tensor_copy(qpk[:, sblk], arp[:])

            # rope on packed q_pe (both heads at once; pswap is block-diagonal,
            # cos/sin tables stacked twice on the host)
            for nb in range(N_SB):
                sblk = slice(nb * 512, (nb + 1) * 512)
                cqt = cqts[nb]
                swp2 = ps([P, 512], "qacc", "swp2", bufs=4)
                nc.tensor.matmul(swp2[:], psw[:], qpk[:, sblk], start=True, stop=True)
                sw2 = work.tile([P, 512], F32R, name="sw2")
                nc.vector.tensor_mul(sw2[:], swp2[:], cqt[:, 1, :])
                t2 = work.tile([P, 512], F32R, name="t2")
                nc.vector.tensor_mul(t2[:], qpk[:, sblk], cqt[:, 0, :])
                with nc.allow_low_precision(reason="flash operands are fp16"):
                    nc.vector.tensor_add(qpb[0][:, sblk], t2[:ROPE, :], sw2[:ROPE, :])
                    nc.vector.tensor_add(qpb[1][:, sblk], t2[ROPE:, :], sw2[ROPE:, :])

            # ========= phase 4: flash attention (causal) + o_proj =========
            # Normalization (z matmul -> reciprocal -> broadcast -> scale) and
            # o_proj are emitted one (b,h) iteration behind, right after the
            # next iteration's first score matmuls, so the PE never waits on
            # the Vector reciprocal chain. Row-sums accumulate on Vector.
            # Diagonal key tiles only touch their causally-valid query columns.
            pend_z = []      # z matmul + reciprocal: flushed at iteration start
            pend_n = []      # broadcast + scale + o_proj: flushed two
                             # consume-steps later, once the reciprocal has
                             # had real PE work to hide under

            def flush(q):
                while q:
                    q.pop(0)()

            def make_z(oac, exsum, sblk, h):
                rz = work.tile([1, 512], F32R, name="rz")

                def go_z():
                    zac = ps([1, 512], "zacc", "z_acc")
                    nc.tensor.matmul(zac[:], ones_col[:], exsum[:],
                                     start=True, stop=True)
                    with nc.allow_low_precision(reason="float32r is bitwise float32"):
                        nc.vector.reciprocal(rz[:], zac[:])

                def go_bc():
                    bcz = ps([P, 512], "bcast", "bcz")
                    nc.tensor.matmul(bcz[:], ones_row[:], rz[:], start=True, stop=True)
                    bczs = work.tile([P, 512], F32, name="bczs")
                    nc.vector.tensor_copy(bczs[:], bcz[:])
                    with nc.allow_low_precision(reason="fp16 o_proj operands"):
                        nc.vector.tensor_mul(ao[h][:, sblk], oac[:], bczs[:])
                return go_z, go_bc

            def make_oproj(b):
                def go():
                    for sc in range(4 * b, 4 * b + 4):
                        ssl = slice(sc * P, (sc + 1) * P)
                        for nb2 in range(N_SB):
                            osl = slice(nb2 * 512, (nb2 + 1) * 512)
                            acc = ps([P, 512], "qacc", "oo_acc", bufs=4)
                            nc.tensor.matmul(acc[:], ao[0][:, ssl], wos[:, 0, osl],
                                             start=True, stop=False)
                            nc.tensor.matmul(acc[:], ao[1][:, ssl], wos[:, 1, osl],
                                             start=False, stop=True)
                            ot = work.tile([P, 512], F16, name="ot", bufs=3)
                            with nc.allow_low_precision(reason="fp16 output partials, host-summed in f64"):
                                nc.vector.tensor_copy(ot[:], acc[:])
                            nc.sync.dma_start(yout[ssl, osl], ot[:])
                return go

            for b in range(N_SB):
                for h in range(HPC):
                    sblk = slice(b * 512, (b + 1) * 512)
                    n_tc = 4 * (b + 1)
                    oac = ps([P, 512], "oacc" if h == 0 else "oacc2", "o_acc")
                    exsum = work.tile([P, 512], F32R, name="exsum", bufs=2)
                    exq = []

                    def emit_scores(t):
                        j = t - 4 * b
                        cj = 128 * j if j > 0 else 0
                        ncol = 512 - cj
                        qsl = slice(b * 512 + cj, (b + 1) * 512)
                        tsl = slice(t * P, (t + 1) * P)
                        sacc = ps([P, 512], "qacc", "s_acc", bufs=4)
                        nc.tensor.matmul(sacc[:, cj:], kn[h][:, tsl], qn[h][:, qsl],
                                         start=True, stop=False)
                        nc.tensor.matmul(sacc[:, cj:], kpe_sb[:, tsl], qpb[h][:, qsl],
                                         start=False, stop=True)
                        ex = epool.tile([P, 512], FLASH_DT, name="ex")
                        if cj > 0:
                            nc.vector.memset(ex[:, :cj], 0.0)
                        nc.scalar.activation(ex[:, cj:], sacc[:, cj:], Exp,
                                             scale=SCALE, bias=negc[:])
                        if j >= 0:
                            # causal mask: zero the upper-triangular part of
                            # the diagonal tile on the (idle) gpsimd engine
                            nc.gpsimd.affine_select(
                                out=ex[:, cj:], in_=ex[:, cj:],
                                compare_op=mybir.AluOpType.is_ge, fill=0.0,
                                base=0, pattern=[[1, ncol]],
                                channel_multiplier=-1,
                            )
                        exq.append(ex)

                    def emit_consume(t):
                        ex = exq.pop(0)
                        nc.tensor.matmul(oac[:], vv[:, t, h * VDIM:(h + 1) * VDIM],
                                         ex[:], start=(t == 0), stop=(t == n_tc - 1))
                        if t == 0:
                            nc.gpsimd.tensor_copy(exsum[:], ex[:])
                        else:
                            nc.gpsimd.tensor_add(exsum[:], exsum[:], ex[:])

                    emit_scores(0)
                    emit_scores(1)
                    flush(pend_z)
                    emit_scores(2)
                    for t in range(n_tc):
                        if t + 3 < n_tc:
                            emit_scores(t + 3)
                        emit_consume(t)
                        if t == 1:
                            flush(pend_n)
                    go_z, go_bc = make_z(oac, exsum, sblk, h)
                    pend_z.append(go_z)
                    pend_n.append(go_bc)
                if b > 0:
                    pend_n.append(make_oproj(b - 1))
            flush(pend_z)
            flush(pend_n)
            make_oproj(N_SB - 1)()


_CACHED = None


def _get_program():
    global _CACHED
    if _CACHED is None:
        _CACHED = build_program()
    return _CACHED


def _host_prep(hidden_states, w_qkv_a, q_a_ln_w, w_q_b, w_kv_b, kv_a_ln_w, w_o,
               positions):
    f32 = np.float32
    hs = np.asarray(hidden_states, dtype=f32)
    w1m = np.asarray(w_qkv_a, dtype=f32)
    wqm = np.asarray(w_q_b, dtype=f32) * np.asarray(q_a_ln_w, f32)[None, :]
    wkvm = np.asarray(w_kv_b, dtype=f32) * np.asarray(kv_a_ln_w, f32)[None, :]
    wom = np.asarray(w_o, dtype=f32)

    # rope tables (interleaved / non-neox), matching the reference fp32 math
    pos = np.asarray(positions).astype(f32)
    inv_freq = (1.0 / (f32(THETA) ** (np.arange(0, ROPE, 2, dtype=f32) / f32(ROPE)))).astype(f32)
    fr = pos[None, :] * inv_freq[:, None]              # [32, S]
    cos = np.cos(fr).astype(f32)
    sin = np.sin(fr).astype(f32)
    cosT = np.repeat(cos, 2, axis=0)                   # [64, S]
    ssinT = np.empty((ROPE, SEQ), f32)
    ssinT[0::2] = -sin
    ssinT[1::2] = sin
    # stacked twice: rope tables for the packed two-head q_pe layout
    cos2 = np.ascontiguousarray(np.vstack([cosT, cosT]))     # [128, S]
    ssin2 = np.ascontiguousarray(np.vstack([ssinT, ssinT]))  # [128, S]
    psw1 = np.zeros((ROPE, ROPE), f32)                 # lhsT: out = psw.T @ x
    for i in range(0, ROPE, 2):
        psw1[i + 1, i] = 1.0                           # out[i]   = x[i+1]
        psw1[i, i + 1] = 1.0                           # out[i+1] = x[i]
    psw = np.zeros((P, P), f32)                        # block-diagonal, 2 heads
    psw[:ROPE, :ROPE] = psw1
    psw[ROPE:, ROPE:] = psw1

    hT = hs.T                                          # [I, S]
    # pad w_qkv_a^T out-dim 2112 -> 2176 (17*128); cols past 2112 are zero.
    # One 1MB DMA per output chunk m: w1l[m, p, k*128+j] = w1T[k*128+p, m*128+j]
    # so the (m, k) lhsT block is w1l[m][:, k*128:(k+1)*128].
    w1T = np.zeros((HIDDEN, LAT_C * P), f32)
    w1T[:, :QLR + KVLR + ROPE] = w1m.T
    w1l = np.ascontiguousarray(
        w1T.reshape(N_KC, P, LAT_C, P).transpose(2, 1, 0, 3).reshape(LAT_C, P, HIDDEN)).astype(np.float16)
    wq4 = wqm.reshape(H, QK, QLR)
    wkv4 = wkvm.reshape(H, NOPE + VDIM, KVLR)

    in_maps = []
    for c in range(N_CORES):
        ssl = slice(c * SSH, (c + 1) * SSH)
        h1 = np.ascontiguousarray(hT[:, ssl].reshape(N_KC, P, SSH).transpose(1, 0, 2)).astype(np.float16)
        # column order per k-chunk: [nope_h0 | nope_h1 | rope_h0 | rope_h1]
        wqc2 = wq4[HPC * c:HPC * (c + 1)]                           # [2, 192, QLR]
        wq_cols = np.concatenate([wqc2[0, :NOPE], wqc2[1, :NOPE],
                                  wqc2[0, NOPE:], wqc2[1, NOPE:]], axis=0)  # [384, QLR]
        wqc = wq_cols.T                                             # [QLR, 384]
        wql = np.ascontiguousarray(
            wqc.reshape(N_QAC, P, HPC * QK).transpose(1, 0, 2)).astype(np.float16)
        # column order per k-chunk: [kn_h0 | kn_h1 | v_h0 | v_h1]
        wkvc = wkv4[HPC * c:HPC * (c + 1)]                          # [2, 256, 512]
        wkv_cols = np.concatenate([wkvc[0, :NOPE], wkvc[1, :NOPE],
                                   wkvc[0, NOPE:], wkvc[1, NOPE:]], axis=0)  # [512, KVLR]
        wkvT = wkv_cols.T                                           # [KVLR, 512]
        wkvl = np.ascontiguousarray(
            wkvT.reshape(N_KVC, P, HPC * (NOPE + VDIM)).transpose(1, 0, 2)).astype(np.float16)
        woc = wom[:, HPC * VDIM * c:HPC * VDIM * (c + 1)].T          # [256, 2048]
        wol = np.ascontiguousarray(
            woc.reshape(HPC, P, HIDDEN).transpose(1, 0, 2)).astype(np.float16)
        in_maps.append({
            "h1": h1, "w1": w1l, "wq": wql, "wkv": wkvl, "wo": wol,
            "cosq": cos2.astype(np.float16), "ssinq": ssin2.astype(np.float16),
            "cosl": np.ascontiguousarray(cosT[:, ssl]).astype(np.float16),
            "ssinl": np.ascontiguousarray(ssinT[:, ssl]).astype(np.float16),
            "pswap": psw.astype(np.float16),
            "onesc": np.ones((P, 1), f32),
            "onesr": np.ones((1, P), f32),
        })
    return in_maps


def kernel(**inputs):
    nc = _get_program()
    in_maps = _host_prep(**inputs)
    res = run_bass_kernel_spmd(nc, in_maps, list(range(N_CORES)))
    out = np.zeros((SEQ, HIDDEN), np.float64)
    for c in range(N_CORES):
        out += res.results[c]["y"].astype(np.float64)
    return out.astype(np.float32)
